# revision 1
# baseline (speedup 1.0000x reference)
import numpy as np
import jax
import jax.numpy as jnp
from jax import lax
from functools import partial

BN_EPS = 1e-5
NORM_EPS = 1e-8
N_CORES = 8
B, C, H, W = 64, 3, 512, 512


def _dft_mats():
    # DFT matrices with fftshift folded into the output (row) index:
    # Zs = fftshift(W @ G @ W) row/col permuted. W[k,n] = exp(-2i pi k n / N).
    n = np.arange(H)
    k = (n + H // 2) % H  # inverse-shift: output row j of shifted = row k[j] of unshifted
    ang = -2.0 * np.pi * np.outer(n, n) / H
    Cm = np.cos(ang).astype(np.float32)
    Sm = np.sin(ang).astype(np.float32)
    # shifted versions: Cs[j, n] = C[(j + 256) % 512, n]
    Cs = Cm[k, :]
    Ss = Sm[k, :]
    return Cm, Sm, Cs, Ss


_CM, _SM, _CS, _SS = _dft_mats()


def _conv(x, w, b, stride, pad):
    y = lax.conv_general_dilated(x, w, (stride, stride), [(pad, pad), (pad, pad)],
                                 dimension_numbers=('NCHW', 'OIHW', 'NCHW'))
    return y + b[None, :, None, None]


def _bn(x, g, b, m, v):
    inv = g / jnp.sqrt(v + BN_EPS)
    return x * inv[None, :, None, None] + (b - m * inv)[None, :, None, None]


def _maxpool2(x):
    return lax.reduce_window(x, -jnp.inf, lax.max, (1, 1, 2, 2), (1, 1, 2, 2), 'VALID')


def _branch(x, params):
    (conv1_w, conv1_b, bn1_g, bn1_b, bn1_m, bn1_v,
     conv2_w, conv2_b, bn2_g, bn2_b, bn2_m, bn2_v,
     conv3_w, conv3_b, bn3_g, bn3_b, bn3_m, bn3_v,
     fc1_w, fc1_b, fc2_w, fc2_b, Cm, Sm, Cs, Ss) = params
    gray = 0.299 * x[:, 0] + 0.587 * x[:, 1] + 0.114 * x[:, 2]  # (b,512,512)
    # 2D DFT via matmuls, fftshift folded into row-permuted matrices.
    # Stage 1 (rows / first axis), with shifted output rows:
    U = jnp.einsum('kn,bnm->bkm', Cs, gray)
    V = jnp.einsum('kn,bnm->bkm', Ss, gray)
    # Stage 2 (cols / second axis), shifted:
    Zre = jnp.einsum('bkm,jm->bkj', U, Cs) - jnp.einsum('bkm,jm->bkj', V, Ss)
    Zim = jnp.einsum('bkm,jm->bkj', U, Ss) + jnp.einsum('bkm,jm->bkj', V, Cs)
    mag = jnp.sqrt(Zre * Zre + Zim * Zim)
    logm = jnp.log1p(mag)
    mu = logm.mean(axis=(-2, -1), keepdims=True)
    sd = logm.std(axis=(-2, -1), keepdims=True)
    f = ((logm - mu) / (sd + NORM_EPS))[:, None]
    h = jax.nn.relu(_bn(_conv(f, conv1_w, conv1_b, 2, 2), bn1_g, bn1_b, bn1_m, bn1_v))
    h = _maxpool2(h)
    h = jax.nn.relu(_bn(_conv(h, conv2_w, conv2_b, 2, 1), bn2_g, bn2_b, bn2_m, bn2_v))
    h = _maxpool2(h)
    h = jax.nn.relu(_bn(_conv(h, conv3_w, conv3_b, 2, 1), bn3_g, bn3_b, bn3_m, bn3_v))
    h = h.mean(axis=(-2, -1))
    h = jax.nn.relu(h @ fc1_w.T + fc1_b)
    return jax.nn.relu(h @ fc2_w.T + fc2_b)


_COMPILED = None


def _get_compiled():
    global _COMPILED
    if _COMPILED is None:
        devs = jax.devices()[:N_CORES]
        _COMPILED = jax.pmap(_branch, axis_name='i', devices=devs,
                             in_axes=(0, None))
    return _COMPILED


def kernel(**inputs):
    x = np.asarray(inputs['x'], dtype=np.float32)
    params = tuple(
        jnp.asarray(inputs[k]) for k in (
            'conv1_w', 'conv1_b', 'bn1_g', 'bn1_b', 'bn1_m', 'bn1_v',
            'conv2_w', 'conv2_b', 'bn2_g', 'bn2_b', 'bn2_m', 'bn2_v',
            'conv3_w', 'conv3_b', 'bn3_g', 'bn3_b', 'bn3_m', 'bn3_v',
            'fc1_w', 'fc1_b', 'fc2_w', 'fc2_b')
    ) + (jnp.asarray(_CM), jnp.asarray(_SM), jnp.asarray(_CS), jnp.asarray(_SS))
    xs = x.reshape(N_CORES, B // N_CORES, C, H, W)
    fn = _get_compiled()
    out = fn(xs, params)  # (8, 8, 128)
    return np.asarray(out).reshape(B, 128)


# revision 5
# speedup vs baseline: 85.1783x; 85.1783x over previous
import numpy as np
import jax
import jax.numpy as jnp
from jax import lax
from functools import partial

BN_EPS = 1e-5
NORM_EPS = 1e-8
N_CORES = 8
B, C, H, W = 64, 3, 512, 512
SPLIT_LEVEL = 2  # 1: pure-bf16 DFT matmuls; 2: + hi/lo correction terms


def _np_split(a):
    hi = a.astype(np.float32)
    hi = np.asarray(jnp.asarray(a).astype(jnp.bfloat16))
    lo = np.asarray((jnp.asarray(a) - jnp.asarray(hi).astype(jnp.float32)).astype(jnp.bfloat16))
    return hi, lo


def _dft_mats():
    # DFT matrices with fftshift folded into the output (row) index.
    n = np.arange(H)
    k = (n + H // 2) % H
    ang = -2.0 * np.pi * np.outer(n, n) / H
    Cm = np.cos(ang).astype(np.float32)
    Sm = np.sin(ang).astype(np.float32)
    return Cm[k, :], Sm[k, :]


_CS, _SS = _dft_mats()
_CSH, _CSL = _np_split(_CS)
_SSH, _SSL = _np_split(_SS)


def _conv(x, w, b, stride, pad):
    y = lax.conv_general_dilated(x, w, (stride, stride), [(pad, pad), (pad, pad)],
                                 dimension_numbers=('NCHW', 'OIHW', 'NCHW'))
    return y + b[None, :, None, None]


def _bn(x, g, b, m, v):
    inv = g / jnp.sqrt(v + BN_EPS)
    return x * inv[None, :, None, None] + (b - m * inv)[None, :, None, None]


def _maxpool2(x):
    return lax.reduce_window(x, -jnp.inf, lax.max, (1, 1, 2, 2), (1, 1, 2, 2), 'VALID')


def _branch(x, params):
    (conv1_w, conv1_b, bn1_g, bn1_b, bn1_m, bn1_v,
     conv2_w, conv2_b, bn2_g, bn2_b, bn2_m, bn2_v,
     conv3_w, conv3_b, bn3_g, bn3_b, bn3_m, bn3_v,
     fc1_w, fc1_b, fc2_w, fc2_b) = params
    f32 = jnp.float32
    bf = jnp.bfloat16
    gray = 0.299 * x[:, 0] + 0.587 * x[:, 1] + 0.114 * x[:, 2]  # (b,512,512)

    # DFT constants baked into the executable as bf16 hi/lo pairs.
    Csh, Csl = jnp.asarray(_CSH), jnp.asarray(_CSL)
    Ssh, Ssl = jnp.asarray(_SSH), jnp.asarray(_SSL)

    def split(a):
        hi = a.astype(bf)
        lo = (a - hi.astype(f32)).astype(bf)
        return hi, lo

    def mm(a_hi, a_lo, b_hi, b_lo):
        # contract a dim1 with b dim1; (hi+lo)@(hi+lo) ~= hh [+ hl + lh]
        dn = (((1,), (1,)), ((), ()))
        p = partial(lax.dot_general, dimension_numbers=dn,
                    preferred_element_type=f32)
        acc = p(a_hi, b_hi)
        if SPLIT_LEVEL >= 2:
            acc = acc + p(a_hi, b_lo) + p(a_lo, b_hi)
        return acc

    gh, gl = split(gray)
    U = mm(gh, gl, Csh, Csl)   # (b, m, k): sum_n g[b,n,m] Cs[k,n]
    V = mm(gh, gl, Ssh, Ssl)
    Uh, Ul = split(U)
    Vh, Vl = split(V)
    Zre = mm(Uh, Ul, Csh, Csl) - mm(Vh, Vl, Ssh, Ssl)  # (b, k, j)
    Zim = mm(Uh, Ul, Ssh, Ssl) + mm(Vh, Vl, Csh, Csl)
    mag = jnp.sqrt(Zre * Zre + Zim * Zim)
    logm = jnp.log1p(mag)
    mu = logm.mean(axis=(-2, -1), keepdims=True)
    sd = logm.std(axis=(-2, -1), keepdims=True)
    f = ((logm - mu) / (sd + NORM_EPS))[:, None]
    h = jax.nn.relu(_bn(_conv(f, conv1_w, conv1_b, 2, 2), bn1_g, bn1_b, bn1_m, bn1_v))
    h = _maxpool2(h)
    h = jax.nn.relu(_bn(_conv(h, conv2_w, conv2_b, 2, 1), bn2_g, bn2_b, bn2_m, bn2_v))
    h = _maxpool2(h)
    h = jax.nn.relu(_bn(_conv(h, conv3_w, conv3_b, 2, 1), bn3_g, bn3_b, bn3_m, bn3_v))
    h = h.mean(axis=(-2, -1))
    h = jax.nn.relu(h @ fc1_w.T + fc1_b)
    return jax.nn.relu(h @ fc2_w.T + fc2_b)


_COMPILED = None

_PARAM_KEYS = ('conv1_w', 'conv1_b', 'bn1_g', 'bn1_b', 'bn1_m', 'bn1_v',
               'conv2_w', 'conv2_b', 'bn2_g', 'bn2_b', 'bn2_m', 'bn2_v',
               'conv3_w', 'conv3_b', 'bn3_g', 'bn3_b', 'bn3_m', 'bn3_v',
               'fc1_w', 'fc1_b', 'fc2_w', 'fc2_b')


def _get_compiled():
    global _COMPILED
    if _COMPILED is None:
        devs = jax.devices()[:N_CORES]
        _COMPILED = jax.pmap(_branch, axis_name='i', devices=devs,
                             in_axes=(0, None))
    return _COMPILED


def kernel(**inputs):
    x = np.asarray(inputs['x'], dtype=np.float32)
    params = tuple(jnp.asarray(inputs[k]) for k in _PARAM_KEYS)
    xs = x.reshape(N_CORES, B // N_CORES, C, H, W)
    fn = _get_compiled()
    out = fn(xs, params)  # (8, 8, 128)
    return np.asarray(out).reshape(B, 128)


# revision 8
# speedup vs baseline: 96.2466x; 1.1299x over previous
import numpy as np
import jax
import jax.numpy as jnp
from jax import lax
from functools import partial

BN_EPS = 1e-5
NORM_EPS = 1e-8
N_CORES = 8
B, C, H, W = 64, 3, 512, 512
SPLIT_LEVEL = 2  # 1: pure-bf16 DFT matmuls; 2: + hi/lo correction terms


def _np_split(a):
    hi = np.asarray(jnp.asarray(a).astype(jnp.bfloat16))
    lo = np.asarray((jnp.asarray(a) - jnp.asarray(hi).astype(jnp.float32)).astype(jnp.bfloat16))
    return hi, lo


def _dft_mats():
    # DFT matrices with fftshift folded into the output (row) index.
    n = np.arange(H)
    k = (n + H // 2) % H
    ang = -2.0 * np.pi * np.outer(n, n) / H
    Cm = np.cos(ang).astype(np.float32)
    Sm = np.sin(ang).astype(np.float32)
    return Cm[k, :], Sm[k, :]


_CS, _SS = _dft_mats()
_CSH, _CSL = _np_split(_CS)
_SSH, _SSL = _np_split(_SS)


def _conv(x, w, b, stride, pad):
    y = lax.conv_general_dilated(x, w, (stride, stride), [(pad, pad), (pad, pad)],
                                 dimension_numbers=('NCHW', 'OIHW', 'NCHW'))
    return y + b[None, :, None, None]


def _bn(x, g, b, m, v):
    inv = g / jnp.sqrt(v + BN_EPS)
    return x * inv[None, :, None, None] + (b - m * inv)[None, :, None, None]


def _maxpool2(x):
    return lax.reduce_window(x, -jnp.inf, lax.max, (1, 1, 2, 2), (1, 1, 2, 2), 'VALID')


def _branch(x, params):
    (conv1_w, conv1_b, bn1_g, bn1_b, bn1_m, bn1_v,
     conv2_w, conv2_b, bn2_g, bn2_b, bn2_m, bn2_v,
     conv3_w, conv3_b, bn3_g, bn3_b, bn3_m, bn3_v,
     fc1_w, fc1_b, fc2_w, fc2_b) = params
    f32 = jnp.float32
    bf = jnp.bfloat16
    gray = 0.299 * x[:, 0] + 0.587 * x[:, 1] + 0.114 * x[:, 2]  # (b,512,512)

    # DFT constants baked into the executable as bf16 hi/lo pairs.
    Csh, Csl = jnp.asarray(_CSH), jnp.asarray(_CSL)
    Ssh, Ssl = jnp.asarray(_SSH), jnp.asarray(_SSL)

    def split(a):
        hi = a.astype(bf)
        lo = (a - hi.astype(f32)).astype(bf)
        return hi, lo

    def mm(a_hi, a_lo, b_hi, b_lo):
        # contract a dim1 with b dim1; (hi+lo)@(hi+lo) ~= hh [+ hl + lh]
        dn = (((1,), (1,)), ((), ()))
        p = partial(lax.dot_general, dimension_numbers=dn,
                    preferred_element_type=f32)
        acc = p(a_hi, b_hi)
        if SPLIT_LEVEL >= 2:
            acc = acc + p(a_hi, b_lo) + p(a_lo, b_hi)
        return acc

    gh, gl = split(gray)
    U = mm(gh, gl, Csh, Csl)   # (b, m, k): sum_n g[b,n,m] Cs[k,n]
    V = mm(gh, gl, Ssh, Ssl)
    Uh, Ul = split(U)
    Vh, Vl = split(V)
    Zre = mm(Uh, Ul, Csh, Csl) - mm(Vh, Vl, Ssh, Ssl)  # (b, k, j)
    Zim = mm(Uh, Ul, Ssh, Ssl) + mm(Vh, Vl, Csh, Csl)
    mag = jnp.sqrt(Zre * Zre + Zim * Zim)
    logm = jnp.log1p(mag)
    mu = logm.mean(axis=(-2, -1), keepdims=True)
    sd = logm.std(axis=(-2, -1), keepdims=True)
    f = ((logm - mu) / (sd + NORM_EPS))[:, None]
    h = jax.nn.relu(_bn(_conv(f, conv1_w, conv1_b, 2, 2), bn1_g, bn1_b, bn1_m, bn1_v))
    h = _maxpool2(h)
    h = jax.nn.relu(_bn(_conv(h, conv2_w, conv2_b, 2, 1), bn2_g, bn2_b, bn2_m, bn2_v))
    h = _maxpool2(h)
    h = jax.nn.relu(_bn(_conv(h, conv3_w, conv3_b, 2, 1), bn3_g, bn3_b, bn3_m, bn3_v))
    h = h.mean(axis=(-2, -1))
    h = jax.nn.relu(h @ fc1_w.T + fc1_b)
    return jax.nn.relu(h @ fc2_w.T + fc2_b)


_COMPILED = None

_PARAM_KEYS = ('conv1_w', 'conv1_b', 'bn1_g', 'bn1_b', 'bn1_m', 'bn1_v',
               'conv2_w', 'conv2_b', 'bn2_g', 'bn2_b', 'bn2_m', 'bn2_v',
               'conv3_w', 'conv3_b', 'bn3_g', 'bn3_b', 'bn3_m', 'bn3_v',
               'fc1_w', 'fc1_b', 'fc2_w', 'fc2_b')


def _get_compiled():
    global _COMPILED
    if _COMPILED is None:
        devs = jax.devices()[:N_CORES]
        _COMPILED = jax.pmap(_branch, axis_name='i', devices=devs,
                             in_axes=(0, None))
    return _COMPILED


def kernel(**inputs):
    x = np.asarray(inputs['x'], dtype=np.float32)
    params = tuple(jnp.asarray(inputs[k]) for k in _PARAM_KEYS)
    xs = x.reshape(N_CORES, B // N_CORES, C, H, W)
    fn = _get_compiled()
    out = fn(xs, params)  # (8, 8, 128)
    return np.asarray(out).reshape(B, 128)


# revision 11
# speedup vs baseline: 111.5851x; 1.1594x over previous
import numpy as np
import jax
import jax.numpy as jnp
from jax import lax
from functools import partial

BN_EPS = 1e-5
NORM_EPS = 1e-8
N_CORES = 8
B, C, H, W = 64, 3, 512, 512
SPLIT_LEVEL = 1  # 1: pure-bf16 DFT matmuls; 2: + hi/lo correction terms


def _np_split(a):
    hi = np.asarray(jnp.asarray(a).astype(jnp.bfloat16))
    lo = np.asarray((jnp.asarray(a) - jnp.asarray(hi).astype(jnp.float32)).astype(jnp.bfloat16))
    return hi, lo


def _dft_mats():
    # DFT matrices with fftshift folded into the output (row) index.
    n = np.arange(H)
    k = (n + H // 2) % H
    ang = -2.0 * np.pi * np.outer(n, n) / H
    Cm = np.cos(ang).astype(np.float32)
    Sm = np.sin(ang).astype(np.float32)
    return Cm[k, :], Sm[k, :]


_CS, _SS = _dft_mats()
_CSH, _CSL = _np_split(_CS)
_SSH, _SSL = _np_split(_SS)


def _conv(x, w, b, stride, pad):
    y = lax.conv_general_dilated(x, w, (stride, stride), [(pad, pad), (pad, pad)],
                                 dimension_numbers=('NCHW', 'OIHW', 'NCHW'))
    return y + b[None, :, None, None]


def _bn(x, g, b, m, v):
    inv = g / jnp.sqrt(v + BN_EPS)
    return x * inv[None, :, None, None] + (b - m * inv)[None, :, None, None]


def _maxpool2(x):
    return lax.reduce_window(x, -jnp.inf, lax.max, (1, 1, 2, 2), (1, 1, 2, 2), 'VALID')


def _branch(x, params):
    (conv1_w, conv1_b, bn1_g, bn1_b, bn1_m, bn1_v,
     conv2_w, conv2_b, bn2_g, bn2_b, bn2_m, bn2_v,
     conv3_w, conv3_b, bn3_g, bn3_b, bn3_m, bn3_v,
     fc1_w, fc1_b, fc2_w, fc2_b) = params
    f32 = jnp.float32
    bf = jnp.bfloat16
    gray = 0.299 * x[:, 0] + 0.587 * x[:, 1] + 0.114 * x[:, 2]  # (b,512,512)

    # DFT constants baked into the executable as bf16 hi/lo pairs.
    Csh, Csl = jnp.asarray(_CSH), jnp.asarray(_CSL)
    Ssh, Ssl = jnp.asarray(_SSH), jnp.asarray(_SSL)

    def split(a):
        hi = a.astype(bf)
        lo = (a - hi.astype(f32)).astype(bf)
        return hi, lo

    def mm(a_hi, a_lo, b_hi, b_lo):
        # contract a dim1 with b dim1; (hi+lo)@(hi+lo) ~= hh [+ hl + lh]
        dn = (((1,), (1,)), ((), ()))
        p = partial(lax.dot_general, dimension_numbers=dn,
                    preferred_element_type=f32)
        acc = p(a_hi, b_hi)
        if SPLIT_LEVEL >= 2:
            acc = acc + p(a_hi, b_lo) + p(a_lo, b_hi)
        return acc

    gh, gl = split(gray)
    U = mm(gh, gl, Csh, Csl)   # (b, m, k): sum_n g[b,n,m] Cs[k,n]
    V = mm(gh, gl, Ssh, Ssl)
    Uh, Ul = split(U)
    Vh, Vl = split(V)
    Zre = mm(Uh, Ul, Csh, Csl) - mm(Vh, Vl, Ssh, Ssl)  # (b, k, j)
    Zim = mm(Uh, Ul, Ssh, Ssl) + mm(Vh, Vl, Csh, Csl)
    mag = jnp.sqrt(Zre * Zre + Zim * Zim)
    logm = jnp.log1p(mag)
    mu = logm.mean(axis=(-2, -1), keepdims=True)
    sd = logm.std(axis=(-2, -1), keepdims=True)
    f = ((logm - mu) / (sd + NORM_EPS))[:, None]
    h = jax.nn.relu(_bn(_conv(f, conv1_w, conv1_b, 2, 2), bn1_g, bn1_b, bn1_m, bn1_v))
    h = _maxpool2(h)
    h = jax.nn.relu(_bn(_conv(h, conv2_w, conv2_b, 2, 1), bn2_g, bn2_b, bn2_m, bn2_v))
    h = _maxpool2(h)
    h = jax.nn.relu(_bn(_conv(h, conv3_w, conv3_b, 2, 1), bn3_g, bn3_b, bn3_m, bn3_v))
    h = h.mean(axis=(-2, -1))
    h = jax.nn.relu(h @ fc1_w.T + fc1_b)
    return jax.nn.relu(h @ fc2_w.T + fc2_b)


_COMPILED = None

_PARAM_KEYS = ('conv1_w', 'conv1_b', 'bn1_g', 'bn1_b', 'bn1_m', 'bn1_v',
               'conv2_w', 'conv2_b', 'bn2_g', 'bn2_b', 'bn2_m', 'bn2_v',
               'conv3_w', 'conv3_b', 'bn3_g', 'bn3_b', 'bn3_m', 'bn3_v',
               'fc1_w', 'fc1_b', 'fc2_w', 'fc2_b')


def _get_compiled():
    global _COMPILED
    if _COMPILED is None:
        devs = jax.devices()[:N_CORES]
        _COMPILED = jax.pmap(_branch, axis_name='i', devices=devs,
                             in_axes=(0, None))
    return _COMPILED


def kernel(**inputs):
    x = np.asarray(inputs['x'], dtype=np.float32)
    params = tuple(jnp.asarray(inputs[k]) for k in _PARAM_KEYS)
    xs = x.reshape(N_CORES, B // N_CORES, C, H, W)
    fn = _get_compiled()
    out = fn(xs, params)  # (8, 8, 128)
    return np.asarray(out).reshape(B, 128)
